# revision 21
# baseline (speedup 1.0000x reference)
"""Trainium2 Bass kernel for nn_MinModel_67388036874289.

Model (per batch b):
    e = one_hot(idx[b], V)                       # [T, V]
    a1 = softmax(causal(Toeplitz(v_weight)))     # [T, T], shared across b
    layer_one = a1 @ e                           # [T, V]
    key_t = layer_one @ W.T                      # [T, V]
    a2 = softmax(causal(e @ key_t.T))            # [T, T]
    logits = a2 @ e                              # [T, V]

Key structure exploited: e is one-hot, so
    a2_pre[t, s] = sum_s' a1[s, s'] * W[idx[t], idx[s']]
which needs only a double gather of W (no dense [T,V]x[V,V] matmuls):
    Wr  = W[idx, :]            (row gather,   indirect DMA)
    WrT = Wr.T                 (TensorE transposes, bounced through DRAM)
    MT[s',t] = WrT[idx[s'], t] (row gather,   indirect DMA)

Everything is carried transposed (a2T[s, t]) so no [T,T] transposes are
needed after the matmuls:
    a2T_un   = exp1T.T @ MT    per 128-row s-chunk, K = s' (causal-pruned)
    expA2T   = exp(rinv1[s] * a2T_un), causal-masked to 0  (a1's softmax
               denominator rinv1 folded into the exp scale)
    logits[t, :] = rinv2[t] * (expA2T.T @ E)   (a2's softmax denominator
               rinv2 folded into the PSUM->SBUF output scale)
Softmax denominators along the partition dim come from ones-vector
matmuls on TensorE.

Sharding: data-parallel, one batch per NeuronCore (B == 8 == n_cores).
Self-contained: hardcodes shapes B=8, T=1024, V=2048.
"""

import sys

for _p in ("/opt/trn_rl_repo",):
    if _p not in sys.path:
        sys.path.append(_p)

import numpy as np
import ml_dtypes

import concourse.bass as bass
import concourse.mybir as mybir
import concourse.tile as tile
from concourse.bass_utils import run_bass_kernel_spmd
from concourse.tile import TileContext
from concourse.masks import make_identity
from concourse.vector_clock import ScopedClock

B, T, V = 8, 1024, 2048
P = 128
NT = T // P   # 8 tiles along T
NV = V // P   # 16 tiles along V
F32 = mybir.dt.float32
BF16 = mybir.dt.bfloat16
I32 = mybir.dt.int32
I16 = mybir.dt.int16

# ---------------------------------------------------------------------------
# Walrus in this environment rejects >1 semaphore wait per instruction
# ("Too many sync wait commands"). Spill extra waits onto same-engine NoOps.
# ---------------------------------------------------------------------------
_MAX_WAITS = 1
_patch_done = False


def _install_tile_patch():
    global _patch_done
    if _patch_done:
        return
    _patch_done = True

    _orig_lower = TileContext._lower_ordered_insts

    def _split(self, ordered):
        for _bb, insts in ordered.items():
            i = 0
            while i < len(insts):
                inst = insts[i]
                si = getattr(inst, "sync_info", None)
                if si is not None and si.on_wait and len(si.on_wait) > _MAX_WAITS:
                    waits = list(si.on_wait)
                    inst.sync_info = mybir.SyncInfo(
                        on_wait=waits[:_MAX_WAITS], on_update=list(si.on_update)
                    )
                    spills = [
                        mybir.InstNoOp(
                            name=self.nc.get_next_instruction_name(),
                            sync_info=mybir.SyncInfo(
                                on_wait=waits[j : j + _MAX_WAITS], on_update=[]
                            ),
                            bass_nofuse=True,
                            engine=inst.engine,
                        )
                        for j in range(_MAX_WAITS, len(waits), _MAX_WAITS)
                    ]
                    insts[i:i] = spills
                    i += len(spills)
                i += 1

    def _patched_lower(self, ordered):
        _split(self, ordered)
        return _orig_lower(self, ordered)

    def _patched_drain_and_barrier(self, tick_clock, wait_clock):
        nc = self.nc
        carrier = nc.sync.nop(nofuse=True)
        wait_clock.add_sem_waits(
            carrier.ins, ScopedClock({None: tick_clock.global_clock})
        )
        si = carrier.ins.sync_info
        if si is not None and len(si.on_wait) > _MAX_WAITS:
            waits = list(si.on_wait)
            carrier.ins.sync_info = mybir.SyncInfo(
                on_wait=waits[:_MAX_WAITS], on_update=list(si.on_update)
            )
            for i in range(_MAX_WAITS, len(waits), _MAX_WAITS):
                extra = nc.sync.nop(nofuse=True)
                extra.ins.sync_info = mybir.SyncInfo(
                    on_wait=waits[i : i + _MAX_WAITS], on_update=[]
                )
        nc.sync.drain()
        nc.all_engine_barrier()
        assert self.sems is not None
        popped = nc._tile_sem_poison_stack.pop()
        assert popped is self._sem_poison
        nc.clear_and_free_semaphores(list(self.sems.allocated().values()))
        nc.all_engine_barrier()

    TileContext._lower_ordered_insts = _patched_lower
    TileContext._drain_and_barrier = _patched_drain_and_barrier


# ---------------------------------------------------------------------------
# Kernel build
# ---------------------------------------------------------------------------
_nc_cache = None


def _build():
    global _nc_cache
    if _nc_cache is not None:
        return _nc_cache
    _install_tile_patch()

    nc = bass.Bass(num_devices=B)
    idx_d = nc.declare_dram_parameter("idx", [NT, P], I32, isOutput=False)
    # a1rawT[k, q] = v_weight[q - k] for k <= q else -1e30 (host Toeplitz)
    a1rawt_d = nc.declare_dram_parameter("a1rawt", [T, T], BF16, isOutput=False)
    w_d = nc.declare_dram_parameter("W", [V, V], BF16, isOutput=False)
    out_d = nc.declare_dram_parameter("out", [T, V], F32, isOutput=True)

    wrt_dram = nc.dram_tensor("wrt_scratch", [V, T], BF16)
    rs1_dram = nc.dram_tensor("rs1_scratch", [T], F32)

    with TileContext(nc) as tc:
        with (
            tc.tile_pool(name="const", bufs=1) as cpool,
            tc.tile_pool(name="persist", bufs=1) as pers,
            tc.tile_pool(name="work", bufs=3) as work,
            tc.tile_pool(name="stats", bufs=1) as stats,
            tc.tile_pool(name="wrk1", bufs=4) as wrk1,
            tc.tile_pool(name="pst", bufs=2, space="PSUM") as pst,
            tc.tile_pool(name="psmm", bufs=5, space="PSUM") as psmm,
            tc.tile_pool(name="pssm", bufs=1, space="PSUM") as pssm,
        ):
            # ---- constants ----
            ident_f = cpool.tile([P, P], F32)
            make_identity(nc, ident_f[:])
            ident_b = cpool.tile([P, P], BF16)
            nc.vector.tensor_copy(out=ident_b[:], in_=ident_f[:])

            idx_sb = cpool.tile([P, NT], I32)
            nc.sync.dma_start(out=idx_sb[:], in_=idx_d.rearrange("a p -> p a"))
            idx16 = cpool.tile([P, NT], I16)
            nc.vector.tensor_copy(out=idx16[:], in_=idx_sb[:])

            iota = cpool.tile([P, V], I16)
            nc.gpsimd.iota(
                out=iota[:], pattern=[[1, V]], base=0, channel_multiplier=0
            )
            ones_col = cpool.tile([P, 1], BF16)
            nc.gpsimd.memset(ones_col[:], 1.0)

            # ---- persistent arrays ----
            wrt = [pers.tile([P, T], BF16, name=f"wrt{vj}", tag=f"wrt{vj}") for vj in range(NV)]
            mt = [pers.tile([P, T], BF16, name=f"mt{si}", tag=f"mt{si}") for si in range(NT)]
            e1t = [pers.tile([P, T], BF16, name=f"e1t{si}", tag=f"e1t{si}") for si in range(NT)]
            E = [pers.tile([P, V], BF16, name=f"E{si}", tag=f"E{si}") for si in range(NT)]
            ea2 = [pers.tile([P, T], BF16, name=f"ea2{si}", tag=f"ea2{si}") for si in range(NT)]
            rinv1 = cpool.tile([P, NT], F32)
            rinv2 = [stats.tile([P, 1], F32, name=f"rinv2{ti}", tag=f"rinv2{ti}") for ti in range(NT)]

            # ---- phase A: gather W rows (bf16), transpose via matmuls ----
            with tc.tile_pool(name="wrpool", bufs=3) as wrpool:
                for ti in range(NT):
                    wr = wrpool.tile([P, V], BF16, tag="wr")
                    nc.gpsimd.indirect_dma_start(
                        out=wr[:],
                        out_offset=None,
                        in_=w_d[:, :],
                        in_offset=bass.IndirectOffsetOnAxis(
                            ap=idx_sb[:, ti : ti + 1], axis=0
                        ),
                    )
                    # transpose 16 blocks as regular matmuls (out = blk.T @ I),
                    # 4 blocks share one PSUM bank
                    for vq in range(NV // 4):
                        ptw = pst.tile([P, 512], F32, space="PSUM", tag="ptw")
                        for k in range(4):
                            vj = vq * 4 + k
                            nc.tensor.matmul(
                                out=ptw[:, k * P : (k + 1) * P],
                                lhsT=wr[:, vj * P : (vj + 1) * P],
                                rhs=ident_b[:],
                                start=True,
                                stop=True,
                            )
                        for k in range(4):
                            vj = vq * 4 + k
                            nc.any.tensor_copy(
                                out=wrt[vj][:, ti * P : (ti + 1) * P],
                                in_=ptw[:, k * P : (k + 1) * P],
                            )
                    # after the left T-half is transposed, ship it out so the
                    # second gather's left half can start early
                    if ti == 3:
                        for vj in range(NV):
                            nc.sync.dma_start(
                                out=wrt_dram[vj * P : (vj + 1) * P, 0:512],
                                in_=wrt[vj][:, 0:512],
                            )
                for vj in range(NV):
                    nc.sync.dma_start(
                        out=wrt_dram[vj * P : (vj + 1) * P, 512:T],
                        in_=wrt[vj][:, 512:T],
                    )

            # ---- phase A2: exp of transposed Toeplitz (a1 numerators) ----
            for si in range(NT):
                raw = work.tile([P, T], BF16, tag="a1raw")
                nc.sync.dma_start(
                    out=raw[:], in_=a1rawt_d[si * P : (si + 1) * P, :]
                )
                nc.scalar.activation(
                    out=e1t[si][:],
                    in_=raw[:],
                    func=mybir.ActivationFunctionType.Exp,
                )

            # ---- phase A3: one-hot E tiles (int16 compare) ----
            for si in range(NT):
                nc.vector.tensor_tensor(
                    out=E[si][:],
                    in0=iota[:],
                    in1=idx16[:, si : si + 1].to_broadcast([P, V]),
                    op=mybir.AluOpType.is_equal,
                )

            # ---- a1 softmax denominators: rowsum1[q] = sum_k exp1T[k, q] --
            rs1 = cpool.tile([1, T], F32)
            for tj in range(2):
                psr = pssm.tile([1, 512], F32, space="PSUM", tag="psr")
                klist = list(range(min(4 * (tj + 1), NT)))
                for n, si in enumerate(klist):
                    nc.tensor.matmul(
                        out=psr[:],
                        lhsT=ones_col[:],
                        rhs=e1t[si][:, tj * 512 : (tj + 1) * 512],
                        start=(n == 0),
                        stop=(n == len(klist) - 1),
                    )
                nc.any.tensor_copy(out=rs1[:, tj * 512 : (tj + 1) * 512], in_=psr[:])
            # columnize [1, 1024] -> [128, 8] with a DRAM bounce
            nc.sync.dma_start(out=rs1_dram[None, :], in_=rs1[0:1, :])
            rs1c = cpool.tile([P, NT], F32)
            nc.sync.dma_start(
                out=rs1c[:], in_=rs1_dram.rearrange("(a p) -> p a", p=P)
            )
            nc.vector.reciprocal(out=rinv1[:], in_=rs1c[:])

            # ---- phase B: second gather (T-halves so phase C starts early)
            for half in range(2):
                tsl = slice(half * 512, (half + 1) * 512)
                for si in range(NT):
                    nc.gpsimd.indirect_dma_start(
                        out=mt[si][:, tsl],
                        out_offset=None,
                        in_=wrt_dram[:, :],
                        in_offset=bass.IndirectOffsetOnAxis(
                            ap=idx_sb[:, si : si + 1], axis=0
                        ),
                        element_offset=half * 512,
                    )

            # ---- phases C+D interleaved per s-chunk ----
            # C(si): a2T chunk = exp1T.T @ MT, fused exp(scale=rinv1)+mask
            # D(ti=si): logits rows = rinv2 * (expA2T.T @ E) with rowsum2
            #           accumulated from the unnormalized PSUM drains
            for si in range(NT):
                tj_list = [0, 1] if si < 4 else [1]
                for tj in tj_list:
                    ps_a2 = psmm.tile([P, 512], F32, space="PSUM", tag="mm")
                    klist = list(range(si + 1))
                    for n, sk in enumerate(klist):
                        nc.tensor.matmul(
                            out=ps_a2[:],
                            lhsT=e1t[sk][:, si * P : (si + 1) * P],
                            rhs=mt[sk][:, tj * 512 : (tj + 1) * 512],
                            start=(n == 0),
                            stop=(n == len(klist) - 1),
                        )
                    nc.scalar.activation(
                        out=ea2[si][:, tj * 512 : (tj + 1) * 512],
                        in_=ps_a2[:],
                        func=mybir.ActivationFunctionType.Exp,
                        scale=rinv1[:, si : si + 1],
                    )
                    # causal: keep where t >= s, i.e. 512*tj + f - 128*si - p >= 0
                    nc.gpsimd.affine_select(
                        out=ea2[si][:, tj * 512 : (tj + 1) * 512],
                        in_=ea2[si][:, tj * 512 : (tj + 1) * 512],
                        compare_op=mybir.AluOpType.is_ge,
                        fill=0.0,
                        base=512 * tj - 128 * si,
                        pattern=[[1, 512]],
                        channel_multiplier=-1,
                    )

                # ---- phase D for ti = si (all ea2[sk<=ti] now ready) ----
                ti = si
                orow = work.tile([P, V], F32, tag="orow")
                s2p = [wrk1.tile([P, 1], F32, name=f"s2p{vj}", tag=f"s2p{vj}") for vj in range(4)]
                for vj in range(4):
                    ps3 = psmm.tile([P, 512], F32, space="PSUM", tag="mm")
                    for sk in range(ti + 1):
                        nc.tensor.matmul(
                            out=ps3[:],
                            lhsT=ea2[sk][:, ti * P : (ti + 1) * P],
                            rhs=E[sk][:, vj * 512 : (vj + 1) * 512],
                            start=(sk == 0),
                            stop=(sk == ti),
                        )
                    # drain with free-dim row-sum accumulation (rowsum2 part)
                    if vj % 2 == 0:
                        nc.scalar.activation(
                            out=orow[:, vj * 512 : (vj + 1) * 512],
                            in_=ps3[:],
                            func=mybir.ActivationFunctionType.Copy,
                            accum_out=s2p[vj][:],
                        )
                    else:
                        nc.vector.tensor_scalar(
                            out=orow[:, vj * 512 : (vj + 1) * 512],
                            in0=ps3[:],
                            scalar1=1.0,
                            scalar2=0.0,
                            op0=mybir.AluOpType.mult,
                            op1=mybir.AluOpType.add,
                            accum_out=s2p[vj][:],
                        )
                sAB = wrk1.tile([P, 1], F32, tag="sAB")
                sCD = wrk1.tile([P, 1], F32, tag="sCD")
                nc.vector.tensor_add(out=sAB[:], in0=s2p[0][:], in1=s2p[1][:])
                nc.vector.tensor_add(out=sCD[:], in0=s2p[2][:], in1=s2p[3][:])
                stot = wrk1.tile([P, 1], F32, tag="stot")
                nc.vector.tensor_add(out=stot[:], in0=sAB[:], in1=sCD[:])
                nc.vector.reciprocal(out=rinv2[ti][:], in_=stot[:])
                nc.vector.tensor_scalar_mul(orow[:], orow[:], rinv2[ti][:, :1])
                nc.sync.dma_start(
                    out=out_d[ti * P : (ti + 1) * P, :], in_=orow[:]
                )

    _nc_cache = nc
    return nc


def _host_a1rawt(v_weight: np.ndarray) -> np.ndarray:
    """Transposed Toeplitz of v_weight with -1e30 on the masked part.

    a1rawT[k, q] = v_weight[q - k] for k <= q, else -1e30. Pure data
    rearrangement of the v_weight input; all arithmetic stays on device.
    """
    v = np.asarray(v_weight, dtype=np.float32).reshape(-1)
    q = np.arange(T)
    relmat = np.clip(q[None, :] - q[:, None], 0, None)
    raw = v[relmat]
    raw[q[:, None] > q[None, :]] = -1e30
    return raw.astype(ml_dtypes.bfloat16)


def kernel(idx, v_weight, W):
    nc = _build()
    idx_np = np.asarray(idx).astype(np.int32)
    w_np = np.ascontiguousarray(
        np.asarray(W, dtype=np.float32).astype(ml_dtypes.bfloat16)
    )
    a1rawt = _host_a1rawt(v_weight)

    in_maps = [
        {
            "idx": np.ascontiguousarray(idx_np[b].reshape(NT, P)),
            "a1rawt": a1rawt,
            "W": w_np,
        }
        for b in range(B)
    ]
    res = run_bass_kernel_spmd(nc, in_maps, list(range(B)))
    return np.stack([np.asarray(res.results[b]["out"]) for b in range(B)], axis=0)


# revision 22
# speedup vs baseline: 1.0446x; 1.0446x over previous
"""Trainium2 Bass kernel for nn_MinModel_67388036874289.

Model (per batch b):
    e = one_hot(idx[b], V)                       # [T, V]
    a1 = softmax(causal(Toeplitz(v_weight)))     # [T, T], shared across b
    layer_one = a1 @ e                           # [T, V]
    key_t = layer_one @ W.T                      # [T, V]
    a2 = softmax(causal(e @ key_t.T))            # [T, T]
    logits = a2 @ e                              # [T, V]

Key structure exploited: e is one-hot, so
    a2_pre[t, s] = sum_s' a1[s, s'] * W[idx[t], idx[s']]
which needs only a double gather of W (no dense [T,V]x[V,V] matmuls):
    Wr  = W[idx, :]            (row gather,   indirect DMA)
    WrT = Wr.T                 (TensorE transposes, bounced through DRAM)
    MT[s',t] = WrT[idx[s'], t] (row gather,   indirect DMA)

Everything is carried transposed (a2T[s, t]) so no [T,T] transposes are
needed after the matmuls:
    a2T_un   = exp1T.T @ MT    per 128-row s-chunk, K = s' (causal-pruned)
    expA2T   = exp(rinv1[s] * a2T_un), causal-masked to 0  (a1's softmax
               denominator rinv1 folded into the exp scale)
    logits[t, :] = rinv2[t] * (expA2T.T @ E)   (a2's softmax denominator
               rinv2 folded into the PSUM->SBUF output scale)
Softmax denominators along the partition dim come from ones-vector
matmuls on TensorE.

Sharding: data-parallel, one batch per NeuronCore (B == 8 == n_cores).
Self-contained: hardcodes shapes B=8, T=1024, V=2048.
"""

import sys

for _p in ("/opt/trn_rl_repo",):
    if _p not in sys.path:
        sys.path.append(_p)

import numpy as np
import ml_dtypes

import concourse.bass as bass
import concourse.mybir as mybir
import concourse.tile as tile
from concourse.bass_utils import run_bass_kernel_spmd
from concourse.tile import TileContext
from concourse.masks import make_identity
from concourse.vector_clock import ScopedClock

B, T, V = 8, 1024, 2048
P = 128
NT = T // P   # 8 tiles along T
NV = V // P   # 16 tiles along V
F32 = mybir.dt.float32
BF16 = mybir.dt.bfloat16
I32 = mybir.dt.int32
I16 = mybir.dt.int16

# ---------------------------------------------------------------------------
# Walrus in this environment rejects >1 semaphore wait per instruction
# ("Too many sync wait commands"). Spill extra waits onto same-engine NoOps.
# ---------------------------------------------------------------------------
_MAX_WAITS = 1
_patch_done = False


def _install_tile_patch():
    global _patch_done
    if _patch_done:
        return
    _patch_done = True

    _orig_lower = TileContext._lower_ordered_insts

    def _split(self, ordered):
        for _bb, insts in ordered.items():
            i = 0
            while i < len(insts):
                inst = insts[i]
                si = getattr(inst, "sync_info", None)
                if si is not None and si.on_wait and len(si.on_wait) > _MAX_WAITS:
                    waits = list(si.on_wait)
                    inst.sync_info = mybir.SyncInfo(
                        on_wait=waits[:_MAX_WAITS], on_update=list(si.on_update)
                    )
                    spills = [
                        mybir.InstNoOp(
                            name=self.nc.get_next_instruction_name(),
                            sync_info=mybir.SyncInfo(
                                on_wait=waits[j : j + _MAX_WAITS], on_update=[]
                            ),
                            bass_nofuse=True,
                            engine=inst.engine,
                        )
                        for j in range(_MAX_WAITS, len(waits), _MAX_WAITS)
                    ]
                    insts[i:i] = spills
                    i += len(spills)
                i += 1

    def _patched_lower(self, ordered):
        _split(self, ordered)
        return _orig_lower(self, ordered)

    def _patched_drain_and_barrier(self, tick_clock, wait_clock):
        nc = self.nc
        carrier = nc.sync.nop(nofuse=True)
        wait_clock.add_sem_waits(
            carrier.ins, ScopedClock({None: tick_clock.global_clock})
        )
        si = carrier.ins.sync_info
        if si is not None and len(si.on_wait) > _MAX_WAITS:
            waits = list(si.on_wait)
            carrier.ins.sync_info = mybir.SyncInfo(
                on_wait=waits[:_MAX_WAITS], on_update=list(si.on_update)
            )
            for i in range(_MAX_WAITS, len(waits), _MAX_WAITS):
                extra = nc.sync.nop(nofuse=True)
                extra.ins.sync_info = mybir.SyncInfo(
                    on_wait=waits[i : i + _MAX_WAITS], on_update=[]
                )
        nc.sync.drain()
        nc.all_engine_barrier()
        assert self.sems is not None
        popped = nc._tile_sem_poison_stack.pop()
        assert popped is self._sem_poison
        nc.clear_and_free_semaphores(list(self.sems.allocated().values()))
        nc.all_engine_barrier()

    TileContext._lower_ordered_insts = _patched_lower
    TileContext._drain_and_barrier = _patched_drain_and_barrier


# ---------------------------------------------------------------------------
# Kernel build
# ---------------------------------------------------------------------------
_nc_cache = None


def _build():
    global _nc_cache
    if _nc_cache is not None:
        return _nc_cache
    _install_tile_patch()

    nc = bass.Bass(num_devices=B)
    idx_d = nc.declare_dram_parameter("idx", [NT, P], I32, isOutput=False)
    # a1rawT[k, q] = v_weight[q - k] for k <= q else -1e30 (host Toeplitz)
    a1rawt_d = nc.declare_dram_parameter("a1rawt", [T, T], BF16, isOutput=False)
    w_d = nc.declare_dram_parameter("W", [V, V], BF16, isOutput=False)
    out_d = nc.declare_dram_parameter("out", [T, V], F32, isOutput=True)

    wrt_dram = nc.dram_tensor("wrt_scratch", [V, T], BF16)
    rs1_dram = nc.dram_tensor("rs1_scratch", [T], F32)

    with TileContext(nc) as tc:
        with (
            tc.tile_pool(name="const", bufs=1) as cpool,
            tc.tile_pool(name="persist", bufs=1) as pers,
            tc.tile_pool(name="work", bufs=3) as work,
            tc.tile_pool(name="stats", bufs=1) as stats,
            tc.tile_pool(name="wrk1", bufs=4) as wrk1,
            tc.tile_pool(name="pst", bufs=2, space="PSUM") as pst,
            tc.tile_pool(name="psmm", bufs=5, space="PSUM") as psmm,
            tc.tile_pool(name="pssm", bufs=1, space="PSUM") as pssm,
        ):
            # ---- constants ----
            ident_f = cpool.tile([P, P], F32)
            make_identity(nc, ident_f[:])
            ident_b = cpool.tile([P, P], BF16)
            nc.vector.tensor_copy(out=ident_b[:], in_=ident_f[:])

            idx_sb = cpool.tile([P, NT], I32)
            nc.sync.dma_start(out=idx_sb[:], in_=idx_d.rearrange("a p -> p a"))
            idx16 = cpool.tile([P, NT], I16)
            nc.vector.tensor_copy(out=idx16[:], in_=idx_sb[:])

            iota = cpool.tile([P, V], I16)
            nc.gpsimd.iota(
                out=iota[:], pattern=[[1, V]], base=0, channel_multiplier=0
            )
            ones_col = cpool.tile([P, 1], BF16)
            nc.gpsimd.memset(ones_col[:], 1.0)

            # ---- persistent arrays ----
            mtall = pers.tile([P, NT * T], BF16, name="mtall", tag="mtall")
            e1t = [pers.tile([P, T], BF16, name=f"e1t{si}", tag=f"e1t{si}") for si in range(NT)]
            E = [pers.tile([P, V], BF16, name=f"E{si}", tag=f"E{si}") for si in range(NT)]
            ea2 = [pers.tile([P, T], BF16, name=f"ea2{si}", tag=f"ea2{si}") for si in range(NT)]
            rinv1 = cpool.tile([P, NT], F32)
            rinv2 = [stats.tile([P, 1], F32, name=f"rinv2{ti}", tag=f"rinv2{ti}") for ti in range(NT)]

            # ---- phase A: gather W rows (bf16), transpose via matmuls ----
            # wrall[p, a*V + v] = W[idx[128a + p], v]; big batched gathers
            # (2 t-tiles per indirect DMA). wrt_all[p, vj*T + t] = WrT.
            with (
                tc.tile_pool(name="wrpool", bufs=1) as wrpool,
                tc.tile_pool(name="wrtpool", bufs=1) as wrtpool,
            ):
                wrall = wrpool.tile([P, NT * V], BF16, name="wrall", tag="wrall")
                wrt_all = wrtpool.tile([P, NV * T], BF16, name="wrt_all", tag="wrt_all")
                for g in range(4):
                    nc.gpsimd.indirect_dma_start(
                        out=wrall[:, g * 2 * V : (g + 1) * 2 * V],
                        out_offset=None,
                        in_=w_d[:, :],
                        in_offset=bass.IndirectOffsetOnAxis(
                            ap=idx_sb[:, g * 2 : (g + 1) * 2], axis=0
                        ),
                    )
                    for ti in (g * 2, g * 2 + 1):
                        for vq in range(NV // 4):
                            ptw = pst.tile([P, 512], F32, space="PSUM", tag="ptw")
                            for k in range(4):
                                vj = vq * 4 + k
                                nc.tensor.matmul(
                                    out=ptw[:, k * P : (k + 1) * P],
                                    lhsT=wrall[:, ti * V + vj * P : ti * V + (vj + 1) * P],
                                    rhs=ident_b[:],
                                    start=True,
                                    stop=True,
                                )
                            for k in range(4):
                                vj = vq * 4 + k
                                nc.any.tensor_copy(
                                    out=wrt_all[:, vj * T + ti * P : vj * T + (ti + 1) * P],
                                    in_=ptw[:, k * P : (k + 1) * P],
                                )
                    # ship each finished t-half of WrT so the second gather
                    # can begin before the other half is transposed
                    if g % 2 == 1:
                        half = g // 2
                        tsl = slice(half * 512, (half + 1) * 512)
                        nc.scalar.dma_start(
                            out=wrt_dram.rearrange("(c a) t -> a c t", a=P)[:, :, tsl],
                            in_=wrt_all[:].rearrange("p (c t) -> p c t", t=T)[:, :, tsl],
                        )

            # ---- phase A2: exp of transposed Toeplitz (a1 numerators) ----
            for si in range(NT):
                raw = work.tile([P, T], BF16, tag="a1raw")
                nc.scalar.dma_start(
                    out=raw[:], in_=a1rawt_d[si * P : (si + 1) * P, :]
                )
                nc.scalar.activation(
                    out=e1t[si][:],
                    in_=raw[:],
                    func=mybir.ActivationFunctionType.Exp,
                )

            # ---- phase A3: one-hot E tiles (int16 compare) ----
            for si in range(NT):
                nc.vector.tensor_tensor(
                    out=E[si][:],
                    in0=iota[:],
                    in1=idx16[:, si : si + 1].to_broadcast([P, V]),
                    op=mybir.AluOpType.is_equal,
                )

            # ---- a1 softmax denominators: rowsum1[q] = sum_k exp1T[k, q] --
            rs1 = cpool.tile([1, T], F32)
            for tj in range(2):
                psr = pssm.tile([1, 512], F32, space="PSUM", tag="psr")
                klist = list(range(min(4 * (tj + 1), NT)))
                for n, si in enumerate(klist):
                    nc.tensor.matmul(
                        out=psr[:],
                        lhsT=ones_col[:],
                        rhs=e1t[si][:, tj * 512 : (tj + 1) * 512],
                        start=(n == 0),
                        stop=(n == len(klist) - 1),
                    )
                nc.any.tensor_copy(out=rs1[:, tj * 512 : (tj + 1) * 512], in_=psr[:])
            # columnize [1, 1024] -> [128, 8] with a DRAM bounce
            nc.sync.dma_start(out=rs1_dram[None, :], in_=rs1[0:1, :])
            rs1c = cpool.tile([P, NT], F32)
            nc.sync.dma_start(
                out=rs1c[:], in_=rs1_dram.rearrange("(a p) -> p a", p=P)
            )
            nc.vector.reciprocal(out=rinv1[:], in_=rs1c[:])

            # ---- phase B: second gather (2 big indirect DMAs) ----
            for h in range(2):
                nc.gpsimd.indirect_dma_start(
                    out=mtall[:, h * 4 * T : (h + 1) * 4 * T],
                    out_offset=None,
                    in_=wrt_dram[:, :],
                    in_offset=bass.IndirectOffsetOnAxis(
                        ap=idx_sb[:, h * 4 : (h + 1) * 4], axis=0
                    ),
                )

            # ---- phases C+D interleaved per s-chunk ----
            # C(si): a2T chunk = exp1T.T @ MT, fused exp(scale=rinv1)+mask
            # D(ti=si): logits rows = rinv2 * (expA2T.T @ E) with rowsum2
            #           accumulated from the unnormalized PSUM drains
            for si in range(NT):
                tj_list = [0, 1] if si < 4 else [1]
                for tj in tj_list:
                    ps_a2 = psmm.tile([P, 512], F32, space="PSUM", tag="mm")
                    klist = list(range(si + 1))
                    for n, sk in enumerate(klist):
                        nc.tensor.matmul(
                            out=ps_a2[:],
                            lhsT=e1t[sk][:, si * P : (si + 1) * P],
                            rhs=mtall[:, sk * T + tj * 512 : sk * T + (tj + 1) * 512],
                            start=(n == 0),
                            stop=(n == len(klist) - 1),
                        )
                    nc.scalar.activation(
                        out=ea2[si][:, tj * 512 : (tj + 1) * 512],
                        in_=ps_a2[:],
                        func=mybir.ActivationFunctionType.Exp,
                        scale=rinv1[:, si : si + 1],
                    )
                    # causal: keep where t >= s, i.e. 512*tj + f - 128*si - p >= 0
                    nc.gpsimd.affine_select(
                        out=ea2[si][:, tj * 512 : (tj + 1) * 512],
                        in_=ea2[si][:, tj * 512 : (tj + 1) * 512],
                        compare_op=mybir.AluOpType.is_ge,
                        fill=0.0,
                        base=512 * tj - 128 * si,
                        pattern=[[1, 512]],
                        channel_multiplier=-1,
                    )

                # ---- phase D for ti = si (all ea2[sk<=ti] now ready) ----
                ti = si
                orow = work.tile([P, V], F32, tag="orow")
                s2p = [wrk1.tile([P, 1], F32, name=f"s2p{vj}", tag=f"s2p{vj}") for vj in range(4)]
                for vj in range(4):
                    ps3 = psmm.tile([P, 512], F32, space="PSUM", tag="mm")
                    for sk in range(ti + 1):
                        nc.tensor.matmul(
                            out=ps3[:],
                            lhsT=ea2[sk][:, ti * P : (ti + 1) * P],
                            rhs=E[sk][:, vj * 512 : (vj + 1) * 512],
                            start=(sk == 0),
                            stop=(sk == ti),
                        )
                    # drain with free-dim row-sum accumulation (rowsum2 part)
                    if vj % 2 == 0:
                        nc.scalar.activation(
                            out=orow[:, vj * 512 : (vj + 1) * 512],
                            in_=ps3[:],
                            func=mybir.ActivationFunctionType.Copy,
                            accum_out=s2p[vj][:],
                        )
                    else:
                        nc.vector.tensor_scalar(
                            out=orow[:, vj * 512 : (vj + 1) * 512],
                            in0=ps3[:],
                            scalar1=1.0,
                            scalar2=0.0,
                            op0=mybir.AluOpType.mult,
                            op1=mybir.AluOpType.add,
                            accum_out=s2p[vj][:],
                        )
                sAB = wrk1.tile([P, 1], F32, tag="sAB")
                sCD = wrk1.tile([P, 1], F32, tag="sCD")
                nc.vector.tensor_add(out=sAB[:], in0=s2p[0][:], in1=s2p[1][:])
                nc.vector.tensor_add(out=sCD[:], in0=s2p[2][:], in1=s2p[3][:])
                stot = wrk1.tile([P, 1], F32, tag="stot")
                nc.vector.tensor_add(out=stot[:], in0=sAB[:], in1=sCD[:])
                nc.vector.reciprocal(out=rinv2[ti][:], in_=stot[:])
                nc.vector.tensor_scalar_mul(orow[:], orow[:], rinv2[ti][:, :1])
                nc.sync.dma_start(
                    out=out_d[ti * P : (ti + 1) * P, :], in_=orow[:]
                )

    _nc_cache = nc
    return nc


def _host_a1rawt(v_weight: np.ndarray) -> np.ndarray:
    """Transposed Toeplitz of v_weight with -1e30 on the masked part.

    a1rawT[k, q] = v_weight[q - k] for k <= q, else -1e30. Pure data
    rearrangement of the v_weight input; all arithmetic stays on device.
    """
    v = np.asarray(v_weight, dtype=np.float32).reshape(-1)
    q = np.arange(T)
    relmat = np.clip(q[None, :] - q[:, None], 0, None)
    raw = v[relmat]
    raw[q[:, None] > q[None, :]] = -1e30
    return raw.astype(ml_dtypes.bfloat16)


def kernel(idx, v_weight, W):
    nc = _build()
    idx_np = np.asarray(idx).astype(np.int32)
    w_np = np.ascontiguousarray(
        np.asarray(W, dtype=np.float32).astype(ml_dtypes.bfloat16)
    )
    a1rawt = _host_a1rawt(v_weight)

    in_maps = [
        {
            "idx": np.ascontiguousarray(idx_np[b].reshape(NT, P)),
            "a1rawt": a1rawt,
            "W": w_np,
        }
        for b in range(B)
    ]
    res = run_bass_kernel_spmd(nc, in_maps, list(range(B)))
    return np.stack([np.asarray(res.results[b]["out"]) for b in range(B)], axis=0)


# revision 26
# speedup vs baseline: 1.2332x; 1.1806x over previous
"""Trainium2 Bass kernel for nn_MinModel_67388036874289.

Model (per batch b):
    e = one_hot(idx[b], V)                       # [T, V]
    a1 = softmax(causal(Toeplitz(v_weight)))     # [T, T], shared across b
    layer_one = a1 @ e                           # [T, V]
    key_t = layer_one @ W.T                      # [T, V]
    a2 = softmax(causal(e @ key_t.T))            # [T, T]
    logits = a2 @ e                              # [T, V]

Key structure exploited: e is one-hot, so
    a2_pre[t, s] = sum_s' a1[s, s'] * W[idx[t], idx[s']]
which needs only a double gather of W (no dense [T,V]x[V,V] matmuls):
    Wr  = W[idx, :]            (row gather,   indirect DMA)
    WrT = Wr.T                 (TensorE transposes, bounced through DRAM)
    MT[s',t] = WrT[idx[s'], t] (row gather,   indirect DMA)

Everything is carried transposed (a2T[s, t]) so no [T,T] transposes are
needed after the matmuls:
    a2T_un   = exp1T.T @ MT    per 128-row s-chunk, K = s' (causal-pruned)
    expA2T   = exp(rinv1[s] * a2T_un), causal-masked to 0  (a1's softmax
               denominator rinv1 folded into the exp scale)
    logits[t, :] = rinv2[t] * (expA2T.T @ E)   (a2's softmax denominator
               rinv2 folded into the PSUM->SBUF output scale)
Softmax denominators along the partition dim come from ones-vector
matmuls on TensorE.

Sharding: data-parallel, one batch per NeuronCore (B == 8 == n_cores).
Self-contained: hardcodes shapes B=8, T=1024, V=2048.
"""

import sys

for _p in ("/opt/trn_rl_repo",):
    if _p not in sys.path:
        sys.path.append(_p)

import numpy as np
import ml_dtypes

import concourse.bass as bass
import concourse.mybir as mybir
import concourse.tile as tile
from concourse.bass_utils import run_bass_kernel_spmd
from concourse.tile import TileContext
from concourse.masks import make_identity
from concourse.vector_clock import ScopedClock

B, T, V = 8, 1024, 2048
P = 128
NT = T // P   # 8 tiles along T
NV = V // P   # 16 tiles along V
F32 = mybir.dt.float32
BF16 = mybir.dt.bfloat16
I32 = mybir.dt.int32
I16 = mybir.dt.int16

# ---------------------------------------------------------------------------
# Walrus in this environment rejects >1 semaphore wait per instruction
# ("Too many sync wait commands"). Spill extra waits onto same-engine NoOps.
# ---------------------------------------------------------------------------
_MAX_WAITS = 1
_patch_done = False


def _install_tile_patch():
    global _patch_done
    if _patch_done:
        return
    _patch_done = True

    _orig_lower = TileContext._lower_ordered_insts

    def _split(self, ordered):
        for _bb, insts in ordered.items():
            i = 0
            while i < len(insts):
                inst = insts[i]
                si = getattr(inst, "sync_info", None)
                if si is not None and si.on_wait and len(si.on_wait) > _MAX_WAITS:
                    waits = list(si.on_wait)
                    inst.sync_info = mybir.SyncInfo(
                        on_wait=waits[:_MAX_WAITS], on_update=list(si.on_update)
                    )
                    spills = [
                        mybir.InstNoOp(
                            name=self.nc.get_next_instruction_name(),
                            sync_info=mybir.SyncInfo(
                                on_wait=waits[j : j + _MAX_WAITS], on_update=[]
                            ),
                            bass_nofuse=True,
                            engine=inst.engine,
                        )
                        for j in range(_MAX_WAITS, len(waits), _MAX_WAITS)
                    ]
                    insts[i:i] = spills
                    i += len(spills)
                i += 1

    def _patched_lower(self, ordered):
        _split(self, ordered)
        return _orig_lower(self, ordered)

    def _patched_drain_and_barrier(self, tick_clock, wait_clock):
        nc = self.nc
        carrier = nc.sync.nop(nofuse=True)
        wait_clock.add_sem_waits(
            carrier.ins, ScopedClock({None: tick_clock.global_clock})
        )
        si = carrier.ins.sync_info
        if si is not None and len(si.on_wait) > _MAX_WAITS:
            waits = list(si.on_wait)
            carrier.ins.sync_info = mybir.SyncInfo(
                on_wait=waits[:_MAX_WAITS], on_update=list(si.on_update)
            )
            for i in range(_MAX_WAITS, len(waits), _MAX_WAITS):
                extra = nc.sync.nop(nofuse=True)
                extra.ins.sync_info = mybir.SyncInfo(
                    on_wait=waits[i : i + _MAX_WAITS], on_update=[]
                )
        nc.sync.drain()
        nc.all_engine_barrier()
        assert self.sems is not None
        popped = nc._tile_sem_poison_stack.pop()
        assert popped is self._sem_poison
        nc.clear_and_free_semaphores(list(self.sems.allocated().values()))
        nc.all_engine_barrier()

    TileContext._lower_ordered_insts = _patched_lower
    TileContext._drain_and_barrier = _patched_drain_and_barrier


# ---------------------------------------------------------------------------
# Kernel build
# ---------------------------------------------------------------------------
_nc_cache = None


def _build():
    global _nc_cache
    if _nc_cache is not None:
        return _nc_cache
    _install_tile_patch()

    nc = bass.Bass(num_devices=B)
    idx_d = nc.declare_dram_parameter("idx", [NT, P], I32, isOutput=False)
    # a1rawT[k, q] = v_weight[q - k] for k <= q else -1e30 (host Toeplitz)
    a1rawt_d = nc.declare_dram_parameter("a1rawt", [T, T], BF16, isOutput=False)
    w_d = nc.declare_dram_parameter("W", [V, V], BF16, isOutput=False)
    out_d = nc.declare_dram_parameter("out", [T, V], F32, isOutput=True)

    wrt_dram = nc.dram_tensor("wrt_scratch", [V, T], BF16)
    rs1_dram = nc.dram_tensor("rs1_scratch", [T], F32)

    with TileContext(nc) as tc:
        with (
            tc.tile_pool(name="const", bufs=1) as cpool,
            tc.tile_pool(name="persist", bufs=1) as pers,
            tc.tile_pool(name="work", bufs=2) as work,
            tc.tile_pool(name="stats", bufs=1) as stats,
            tc.tile_pool(name="wrk1", bufs=4) as wrk1,
            tc.tile_pool(name="pst", bufs=3, space="PSUM") as pst,
            tc.tile_pool(name="psmm", bufs=4, space="PSUM") as psmm,
            tc.tile_pool(name="pssm", bufs=1, space="PSUM") as pssm,
        ):
            # ---- constants ----
            ident_f = cpool.tile([P, P], F32)
            make_identity(nc, ident_f[:])
            ident_b = cpool.tile([P, P], BF16)
            nc.vector.tensor_copy(out=ident_b[:], in_=ident_f[:])

            idx_sb = cpool.tile([P, NT], I32)
            nc.sync.dma_start(out=idx_sb[:], in_=idx_d.rearrange("a p -> p a"))
            idx16 = cpool.tile([P, NT], I16)
            nc.vector.tensor_copy(out=idx16[:], in_=idx_sb[:])

            # ---- persistent arrays ----
            mtall = pers.tile([P, NT * T], BF16, name="mtall", tag="mtall")
            e1t = [pers.tile([P, T], BF16, name=f"e1t{si}", tag=f"e1t{si}") for si in range(NT)]
            E = [pers.tile([P, V], BF16, name=f"E{si}", tag=f"E{si}") for si in range(NT)]
            ea2 = [pers.tile([P, T], BF16, name=f"ea2{si}", tag=f"ea2{si}") for si in range(NT)]
            rinv1 = cpool.tile([P, NT], F32)
            rinv2 = [stats.tile([P, 1], F32, name=f"rinv2{ti}", tag=f"rinv2{ti}") for ti in range(NT)]

            # ---- phase A: gather W rows (bf16), transpose via matmuls ----
            # wrall[p, a*V + v] = W[idx[128a + p], v]; big batched gathers
            # (2 t-tiles per indirect DMA). wrt_all[p, vj*T + t] = WrT.
            with (
                tc.tile_pool(name="wrpool", bufs=1) as wrpool,
                tc.tile_pool(name="wrtpool", bufs=1) as wrtpool,
            ):
                wrall = wrpool.tile([P, NT * V], BF16, name="wrall", tag="wrall")
                wrt_all = wrtpool.tile([P, NV * T], BF16, name="wrt_all", tag="wrt_all")
                for g in range(4):
                    nc.gpsimd.indirect_dma_start(
                        out=wrall[:, g * 2 * V : (g + 1) * 2 * V],
                        out_offset=None,
                        in_=w_d[:, :],
                        in_offset=bass.IndirectOffsetOnAxis(
                            ap=idx_sb[:, g * 2 : (g + 1) * 2], axis=0
                        ),
                    )
                    for ti in (g * 2, g * 2 + 1):
                        for vq in range(NV // 4):
                            ptw = pst.tile([P, 512], F32, space="PSUM", tag="ptw")
                            for k in range(4):
                                vj = vq * 4 + k
                                nc.tensor.matmul(
                                    out=ptw[:, k * P : (k + 1) * P],
                                    lhsT=wrall[:, ti * V + vj * P : ti * V + (vj + 1) * P],
                                    rhs=ident_b[:],
                                    start=True,
                                    stop=True,
                                )
                            nc.any.tensor_copy(
                                out=wrt_all[:, vq * 4 * T : (vq + 1) * 4 * T]
                                .rearrange("p (c t) -> p c t", t=T)[
                                    :, :, ti * P : (ti + 1) * P
                                ],
                                in_=ptw[:].rearrange("p (c t) -> p c t", t=P),
                            )
                    # ship each finished t-half of WrT so the second gather
                    # can begin before the other half is transposed
                    if g % 2 == 1:
                        half = g // 2
                        tsl = slice(half * 512, (half + 1) * 512)
                        nc.scalar.dma_start(
                            out=wrt_dram.rearrange("(c a) t -> a c t", a=P)[:, :, tsl],
                            in_=wrt_all[:].rearrange("p (c t) -> p c t", t=T)[:, :, tsl],
                        )

            # ---- phase A2: exp of transposed Toeplitz (a1 numerators) ----
            for si in range(NT):
                raw = work.tile([P, T], BF16, tag="a1raw")
                nc.scalar.dma_start(
                    out=raw[:], in_=a1rawt_d[si * P : (si + 1) * P, :]
                )
                nc.scalar.activation(
                    out=e1t[si][:],
                    in_=raw[:],
                    func=mybir.ActivationFunctionType.Exp,
                )

            # ---- phase B: second gather (2 big indirect DMAs) ----
            for h in range(2):
                nc.gpsimd.indirect_dma_start(
                    out=mtall[:, h * 4 * T : (h + 1) * 4 * T],
                    out_offset=None,
                    in_=wrt_dram[:, :],
                    in_offset=bass.IndirectOffsetOnAxis(
                        ap=idx_sb[:, h * 4 : (h + 1) * 4], axis=0
                    ),
                )

            iota = cpool.tile([P, V], I16)
            nc.gpsimd.iota(
                out=iota[:], pattern=[[1, V]], base=0, channel_multiplier=0
            )
            ones_col = cpool.tile([P, 1], BF16)
            nc.gpsimd.memset(ones_col[:], 1.0)

            # ---- phase A3: one-hot E tiles (int16 compare) ----
            for si in range(NT):
                nc.vector.tensor_tensor(
                    out=E[si][:],
                    in0=iota[:],
                    in1=idx16[:, si : si + 1].to_broadcast([P, V]),
                    op=mybir.AluOpType.is_equal,
                )

            # ---- a1 softmax denominators: rowsum1[q] = sum_k exp1T[k, q] --
            rs1 = cpool.tile([1, T], F32)
            for tj in range(2):
                psr = pssm.tile([1, 512], F32, space="PSUM", tag="psr")
                klist = list(range(min(4 * (tj + 1), NT)))
                for n, si in enumerate(klist):
                    nc.tensor.matmul(
                        out=psr[:],
                        lhsT=ones_col[:],
                        rhs=e1t[si][:, tj * 512 : (tj + 1) * 512],
                        start=(n == 0),
                        stop=(n == len(klist) - 1),
                    )
                nc.any.tensor_copy(out=rs1[:, tj * 512 : (tj + 1) * 512], in_=psr[:])
            # columnize [1, 1024] -> [128, 8] with a DRAM bounce
            nc.sync.dma_start(out=rs1_dram[None, :], in_=rs1[0:1, :])
            rs1c = cpool.tile([P, NT], F32)
            nc.sync.dma_start(
                out=rs1c[:], in_=rs1_dram.rearrange("(a p) -> p a", p=P)
            )
            nc.vector.reciprocal(out=rinv1[:], in_=rs1c[:])

            # ---- phases C+D interleaved per s-chunk ----
            # C(si): a2T chunk = exp1T.T @ MT, fused exp(scale=rinv1)+mask
            # D(ti=si): logits rows = rinv2 * (expA2T.T @ E) with rowsum2
            #           accumulated from the unnormalized PSUM drains
            for si in range(NT):
                tj_list = [0, 1] if si < 4 else [1]
                for tj in tj_list:
                    ps_a2 = psmm.tile([P, 512], F32, space="PSUM", tag="mm")
                    klist = list(range(si + 1))
                    for n, sk in enumerate(klist):
                        nc.tensor.matmul(
                            out=ps_a2[:],
                            lhsT=e1t[sk][:, si * P : (si + 1) * P],
                            rhs=mtall[:, sk * T + tj * 512 : sk * T + (tj + 1) * 512],
                            start=(n == 0),
                            stop=(n == len(klist) - 1),
                        )
                    nc.scalar.activation(
                        out=ea2[si][:, tj * 512 : (tj + 1) * 512],
                        in_=ps_a2[:],
                        func=mybir.ActivationFunctionType.Exp,
                        scale=rinv1[:, si : si + 1],
                    )
                    # causal: keep where t >= s, i.e. 512*tj + f - 128*si - p >= 0
                    nc.gpsimd.affine_select(
                        out=ea2[si][:, tj * 512 : (tj + 1) * 512],
                        in_=ea2[si][:, tj * 512 : (tj + 1) * 512],
                        compare_op=mybir.AluOpType.is_ge,
                        fill=0.0,
                        base=512 * tj - 128 * si,
                        pattern=[[1, 512]],
                        channel_multiplier=-1,
                    )

                # ---- phase D for ti = si (all ea2[sk<=ti] now ready) ----
                ti = si
                orow = work.tile([P, V], F32, tag="orow")
                s2p = [wrk1.tile([P, 1], F32, name=f"s2p{vj}", tag=f"s2p{vj}") for vj in range(4)]
                for vj in range(4):
                    ps3 = psmm.tile([P, 512], F32, space="PSUM", tag="mm")
                    for sk in range(ti + 1):
                        nc.tensor.matmul(
                            out=ps3[:],
                            lhsT=ea2[sk][:, ti * P : (ti + 1) * P],
                            rhs=E[sk][:, vj * 512 : (vj + 1) * 512],
                            start=(sk == 0),
                            stop=(sk == ti),
                        )
                    # drain with free-dim row-sum accumulation (rowsum2 part)
                    if vj % 2 == 0:
                        nc.scalar.activation(
                            out=orow[:, vj * 512 : (vj + 1) * 512],
                            in_=ps3[:],
                            func=mybir.ActivationFunctionType.Copy,
                            accum_out=s2p[vj][:],
                        )
                    else:
                        nc.vector.tensor_scalar(
                            out=orow[:, vj * 512 : (vj + 1) * 512],
                            in0=ps3[:],
                            scalar1=1.0,
                            scalar2=0.0,
                            op0=mybir.AluOpType.mult,
                            op1=mybir.AluOpType.add,
                            accum_out=s2p[vj][:],
                        )
                sAB = wrk1.tile([P, 1], F32, tag="sAB")
                sCD = wrk1.tile([P, 1], F32, tag="sCD")
                nc.vector.tensor_add(out=sAB[:], in0=s2p[0][:], in1=s2p[1][:])
                nc.vector.tensor_add(out=sCD[:], in0=s2p[2][:], in1=s2p[3][:])
                stot = wrk1.tile([P, 1], F32, tag="stot")
                nc.vector.tensor_add(out=stot[:], in0=sAB[:], in1=sCD[:])
                nc.vector.reciprocal(out=rinv2[ti][:], in_=stot[:])
                orow2 = work.tile([P, V], F32, tag="orow2")
                nc.scalar.activation(
                    out=orow2[:],
                    in_=orow[:],
                    func=mybir.ActivationFunctionType.Copy,
                    scale=rinv2[ti][:, :1],
                )
                nc.sync.dma_start(
                    out=out_d[ti * P : (ti + 1) * P, :], in_=orow2[:]
                )

    _nc_cache = nc
    return nc


def _host_a1rawt(v_weight: np.ndarray) -> np.ndarray:
    """Transposed Toeplitz of v_weight with -1e30 on the masked part.

    a1rawT[k, q] = v_weight[q - k] for k <= q, else -1e30. Pure data
    rearrangement of the v_weight input; all arithmetic stays on device.
    """
    v = np.asarray(v_weight, dtype=np.float32).reshape(-1)
    q = np.arange(T)
    relmat = np.clip(q[None, :] - q[:, None], 0, None)
    raw = v[relmat]
    raw[q[:, None] > q[None, :]] = -1e30
    return raw.astype(ml_dtypes.bfloat16)


def kernel(idx, v_weight, W):
    nc = _build()
    idx_np = np.asarray(idx).astype(np.int32)
    w_np = np.ascontiguousarray(
        np.asarray(W, dtype=np.float32).astype(ml_dtypes.bfloat16)
    )
    a1rawt = _host_a1rawt(v_weight)

    in_maps = [
        {
            "idx": np.ascontiguousarray(idx_np[b].reshape(NT, P)),
            "a1rawt": a1rawt,
            "W": w_np,
        }
        for b in range(B)
    ]
    res = run_bass_kernel_spmd(nc, in_maps, list(range(B)))
    return np.stack([np.asarray(res.results[b]["out"]) for b in range(B)], axis=0)


# revision 28
# speedup vs baseline: 1.2767x; 1.0353x over previous
"""Trainium2 Bass kernel for nn_MinModel_67388036874289.

Model (per batch b):
    e = one_hot(idx[b], V)                       # [T, V]
    a1 = softmax(causal(Toeplitz(v_weight)))     # [T, T], shared across b
    layer_one = a1 @ e                           # [T, V]
    key_t = layer_one @ W.T                      # [T, V]
    a2 = softmax(causal(e @ key_t.T))            # [T, T]
    logits = a2 @ e                              # [T, V]

Key structure exploited: e is one-hot, so
    a2_pre[t, s] = sum_s' a1[s, s'] * W[idx[t], idx[s']]
which needs only a double gather of W (no dense [T,V]x[V,V] matmuls):
    Wr  = W[idx, :]            (row gather,   indirect DMA)
    WrT = Wr.T                 (TensorE transposes, bounced through DRAM)
    MT[s',t] = WrT[idx[s'], t] (row gather,   indirect DMA)

Everything is carried transposed (a2T[s, t]) so no [T,T] transposes are
needed after the matmuls:
    a2T_un   = exp1T.T @ MT    per 128-row s-chunk, K = s' (causal-pruned)
    expA2T   = exp(rinv1[s] * a2T_un), causal-masked to 0  (a1's softmax
               denominator rinv1 folded into the exp scale)
    logits[t, :] = rinv2[t] * (expA2T.T @ E)   (a2's softmax denominator
               rinv2 folded into the PSUM->SBUF output scale)
Softmax denominators along the partition dim come from ones-vector
matmuls on TensorE.

Sharding: data-parallel, one batch per NeuronCore (B == 8 == n_cores).
Self-contained: hardcodes shapes B=8, T=1024, V=2048.
"""

import sys

for _p in ("/opt/trn_rl_repo",):
    if _p not in sys.path:
        sys.path.append(_p)

import numpy as np
import ml_dtypes

import concourse.bass as bass
import concourse.mybir as mybir
import concourse.tile as tile
from concourse.bass_utils import run_bass_kernel_spmd
from concourse.tile import TileContext
from concourse.masks import make_identity
from concourse.vector_clock import ScopedClock

B, T, V = 8, 1024, 2048
P = 128
NT = T // P   # 8 tiles along T
NV = V // P   # 16 tiles along V
F32 = mybir.dt.float32
BF16 = mybir.dt.bfloat16
I32 = mybir.dt.int32
I16 = mybir.dt.int16

# ---------------------------------------------------------------------------
# Walrus in this environment rejects >1 semaphore wait per instruction
# ("Too many sync wait commands"). Spill extra waits onto same-engine NoOps.
# ---------------------------------------------------------------------------
_MAX_WAITS = 1
_patch_done = False


def _install_tile_patch():
    global _patch_done
    if _patch_done:
        return
    _patch_done = True

    _orig_lower = TileContext._lower_ordered_insts

    def _split(self, ordered):
        for _bb, insts in ordered.items():
            i = 0
            while i < len(insts):
                inst = insts[i]
                si = getattr(inst, "sync_info", None)
                if si is not None and si.on_wait and len(si.on_wait) > _MAX_WAITS:
                    waits = list(si.on_wait)
                    inst.sync_info = mybir.SyncInfo(
                        on_wait=waits[:_MAX_WAITS], on_update=list(si.on_update)
                    )
                    spills = [
                        mybir.InstNoOp(
                            name=self.nc.get_next_instruction_name(),
                            sync_info=mybir.SyncInfo(
                                on_wait=waits[j : j + _MAX_WAITS], on_update=[]
                            ),
                            bass_nofuse=True,
                            engine=inst.engine,
                        )
                        for j in range(_MAX_WAITS, len(waits), _MAX_WAITS)
                    ]
                    insts[i:i] = spills
                    i += len(spills)
                i += 1

    def _patched_lower(self, ordered):
        _split(self, ordered)
        return _orig_lower(self, ordered)

    def _patched_drain_and_barrier(self, tick_clock, wait_clock):
        nc = self.nc
        carrier = nc.sync.nop(nofuse=True)
        wait_clock.add_sem_waits(
            carrier.ins, ScopedClock({None: tick_clock.global_clock})
        )
        si = carrier.ins.sync_info
        if si is not None and len(si.on_wait) > _MAX_WAITS:
            waits = list(si.on_wait)
            carrier.ins.sync_info = mybir.SyncInfo(
                on_wait=waits[:_MAX_WAITS], on_update=list(si.on_update)
            )
            engines = [nc.sync, nc.vector, nc.scalar, nc.tensor, nc.gpsimd]
            for n, i in enumerate(range(_MAX_WAITS, len(waits), _MAX_WAITS)):
                extra = engines[n % len(engines)].nop(nofuse=True)
                extra.ins.sync_info = mybir.SyncInfo(
                    on_wait=waits[i : i + _MAX_WAITS], on_update=[]
                )
        nc.sync.drain()
        nc.all_engine_barrier()
        assert self.sems is not None
        popped = nc._tile_sem_poison_stack.pop()
        assert popped is self._sem_poison
        nc.clear_and_free_semaphores(list(self.sems.allocated().values()))
        nc.all_engine_barrier()

    TileContext._lower_ordered_insts = _patched_lower
    TileContext._drain_and_barrier = _patched_drain_and_barrier


# ---------------------------------------------------------------------------
# Kernel build
# ---------------------------------------------------------------------------
_nc_cache = None


def _build():
    global _nc_cache
    if _nc_cache is not None:
        return _nc_cache
    _install_tile_patch()

    nc = bass.Bass(num_devices=B)
    idx_d = nc.declare_dram_parameter("idx", [NT, P], I32, isOutput=False)
    # a1rawT[k, q] = v_weight[q - k] for k <= q else -1e30 (host Toeplitz)
    a1rawt_d = nc.declare_dram_parameter("a1rawt", [T, T], BF16, isOutput=False)
    w_d = nc.declare_dram_parameter("W", [V, V], BF16, isOutput=False)
    out_d = nc.declare_dram_parameter("out", [T, V], F32, isOutput=True)

    wrt_dram = nc.dram_tensor("wrt_scratch", [V, T], BF16)
    rs1_dram = nc.dram_tensor("rs1_scratch", [T], F32)

    with TileContext(nc) as tc:
        with (
            tc.tile_pool(name="const", bufs=1) as cpool,
            tc.tile_pool(name="persist", bufs=1) as pers,
            tc.tile_pool(name="work", bufs=2) as work,
            tc.tile_pool(name="stats", bufs=1) as stats,
            tc.tile_pool(name="wrk1", bufs=4) as wrk1,
            tc.tile_pool(name="pst", bufs=3, space="PSUM") as pst,
            tc.tile_pool(name="psmm", bufs=4, space="PSUM") as psmm,
            tc.tile_pool(name="pssm", bufs=1, space="PSUM") as pssm,
        ):
            # ---- constants ----
            ident_f = cpool.tile([P, P], F32)
            make_identity(nc, ident_f[:])
            ident_b = cpool.tile([P, P], BF16)
            nc.vector.tensor_copy(out=ident_b[:], in_=ident_f[:])

            idx_sb = cpool.tile([P, NT], I32)
            nc.sync.dma_start(out=idx_sb[:], in_=idx_d.rearrange("a p -> p a"))
            idx16 = cpool.tile([P, NT], I16)
            nc.vector.tensor_copy(out=idx16[:], in_=idx_sb[:])

            # ---- persistent arrays ----
            mtall = pers.tile([P, NT * T], BF16, name="mtall", tag="mtall")
            e1t = [pers.tile([P, T], BF16, name=f"e1t{si}", tag=f"e1t{si}") for si in range(NT)]
            E = [pers.tile([P, V], BF16, name=f"E{si}", tag=f"E{si}") for si in range(NT)]
            ea2 = [pers.tile([P, T], BF16, name=f"ea2{si}", tag=f"ea2{si}") for si in range(NT)]
            rinv1 = cpool.tile([P, NT], F32)
            rinv2 = [stats.tile([P, 1], F32, name=f"rinv2{ti}", tag=f"rinv2{ti}") for ti in range(NT)]

            # ---- phase A: gather W rows (bf16), transpose via matmuls ----
            # wrall[p, a*V + v] = W[idx[128a + p], v]; big batched gathers
            # (2 t-tiles per indirect DMA). wrt_all[p, vj*T + t] = WrT.
            with (
                tc.tile_pool(name="wrpool", bufs=1) as wrpool,
                tc.tile_pool(name="wrtpool", bufs=1) as wrtpool,
            ):
                wrall = wrpool.tile([P, NT * V], BF16, name="wrall", tag="wrall")
                wrt_all = wrtpool.tile([P, NV * T], BF16, name="wrt_all", tag="wrt_all")
                for g in range(4):
                    nc.gpsimd.indirect_dma_start(
                        out=wrall[:, g * 2 * V : (g + 1) * 2 * V],
                        out_offset=None,
                        in_=w_d[:, :],
                        in_offset=bass.IndirectOffsetOnAxis(
                            ap=idx_sb[:, g * 2 : (g + 1) * 2], axis=0
                        ),
                    )
                    for ti in (g * 2, g * 2 + 1):
                        for vq in range(NV // 4):
                            ptw = pst.tile([P, 512], F32, space="PSUM", tag="ptw")
                            for k in range(4):
                                vj = vq * 4 + k
                                nc.tensor.matmul(
                                    out=ptw[:, k * P : (k + 1) * P],
                                    lhsT=wrall[:, ti * V + vj * P : ti * V + (vj + 1) * P],
                                    rhs=ident_b[:],
                                    start=True,
                                    stop=True,
                                )
                            _ceng = nc.vector if (ti * 4 + vq) % 3 else nc.scalar
                            _cout = wrt_all[:, vq * 4 * T : (vq + 1) * 4 * T].rearrange(
                                "p (c t) -> p c t", t=T
                            )[:, :, ti * P : (ti + 1) * P]
                            _cin = ptw[:].rearrange("p (c t) -> p c t", t=P)
                            if _ceng is nc.vector:
                                nc.vector.tensor_copy(out=_cout, in_=_cin)
                            else:
                                nc.scalar.activation(
                                    out=_cout,
                                    in_=_cin,
                                    func=mybir.ActivationFunctionType.Copy,
                                )
                    # ship each finished t-half of WrT so the second gather
                    # can begin before the other half is transposed
                    if g % 2 == 1:
                        half = g // 2
                        tsl = slice(half * 512, (half + 1) * 512)
                        nc.scalar.dma_start(
                            out=wrt_dram.rearrange("(c a) t -> a c t", a=P)[:, :, tsl],
                            in_=wrt_all[:].rearrange("p (c t) -> p c t", t=T)[:, :, tsl],
                        )

            # ---- phase A2: exp of transposed Toeplitz (a1 numerators) ----
            for si in range(NT):
                raw = work.tile([P, T], BF16, tag="a1raw")
                nc.scalar.dma_start(
                    out=raw[:], in_=a1rawt_d[si * P : (si + 1) * P, :]
                )
                nc.scalar.activation(
                    out=e1t[si][:],
                    in_=raw[:],
                    func=mybir.ActivationFunctionType.Exp,
                )

            ones_col = cpool.tile([P, 1], BF16)
            nc.gpsimd.memset(ones_col[:], 1.0)

            # ---- a1 softmax denominators: rowsum1[q] = sum_k exp1T[k, q] --
            rs1 = cpool.tile([1, T], F32)
            for tj in range(2):
                psr = pssm.tile([1, 512], F32, space="PSUM", tag="psr")
                klist = list(range(min(4 * (tj + 1), NT)))
                for n, si in enumerate(klist):
                    nc.tensor.matmul(
                        out=psr[:],
                        lhsT=ones_col[:],
                        rhs=e1t[si][:, tj * 512 : (tj + 1) * 512],
                        start=(n == 0),
                        stop=(n == len(klist) - 1),
                    )
                nc.any.tensor_copy(out=rs1[:, tj * 512 : (tj + 1) * 512], in_=psr[:])
            # columnize [1, 1024] -> [128, 8] with a DRAM bounce
            nc.scalar.dma_start(out=rs1_dram[None, :], in_=rs1[0:1, :])
            rs1c = cpool.tile([P, NT], F32)
            nc.scalar.dma_start(
                out=rs1c[:], in_=rs1_dram.rearrange("(a p) -> p a", p=P)
            )
            nc.vector.reciprocal(out=rinv1[:], in_=rs1c[:])

            # ---- phase B: second gather (2 big indirect DMAs) ----
            for h in range(2):
                nc.gpsimd.indirect_dma_start(
                    out=mtall[:, h * 4 * T : (h + 1) * 4 * T],
                    out_offset=None,
                    in_=wrt_dram[:, :],
                    in_offset=bass.IndirectOffsetOnAxis(
                        ap=idx_sb[:, h * 4 : (h + 1) * 4], axis=0
                    ),
                )

            iota = cpool.tile([P, V], I16)
            nc.gpsimd.iota(
                out=iota[:], pattern=[[1, V]], base=0, channel_multiplier=0
            )
            # ---- phase A3: one-hot E tiles (int16 compare) ----
            for si in range(NT):
                nc.vector.tensor_tensor(
                    out=E[si][:],
                    in0=iota[:],
                    in1=idx16[:, si : si + 1].to_broadcast([P, V]),
                    op=mybir.AluOpType.is_equal,
                )

            # ---- phases C+D interleaved per s-chunk ----
            # C(si): a2T chunk = exp1T.T @ MT, fused exp(scale=rinv1)+mask
            # D(ti=si): logits rows = rinv2 * (expA2T.T @ E) with rowsum2
            #           accumulated from the unnormalized PSUM drains
            for si in range(NT):
                tj_list = [0, 1] if si < 4 else [1]
                for tj in tj_list:
                    ps_a2 = psmm.tile([P, 512], F32, space="PSUM", tag="mm")
                    klist = list(range(si + 1))
                    for n, sk in enumerate(klist):
                        nc.tensor.matmul(
                            out=ps_a2[:],
                            lhsT=e1t[sk][:, si * P : (si + 1) * P],
                            rhs=mtall[:, sk * T + tj * 512 : sk * T + (tj + 1) * 512],
                            start=(n == 0),
                            stop=(n == len(klist) - 1),
                        )
                    nc.scalar.activation(
                        out=ea2[si][:, tj * 512 : (tj + 1) * 512],
                        in_=ps_a2[:],
                        func=mybir.ActivationFunctionType.Exp,
                        scale=rinv1[:, si : si + 1],
                    )
                    # causal: keep where t >= s, i.e. 512*tj + f - 128*si - p >= 0
                    nc.gpsimd.affine_select(
                        out=ea2[si][:, tj * 512 : (tj + 1) * 512],
                        in_=ea2[si][:, tj * 512 : (tj + 1) * 512],
                        compare_op=mybir.AluOpType.is_ge,
                        fill=0.0,
                        base=512 * tj - 128 * si,
                        pattern=[[1, 512]],
                        channel_multiplier=-1,
                    )

                # ---- phase D for ti = si (all ea2[sk<=ti] now ready) ----
                ti = si
                orow = work.tile([P, V], F32, tag="orow")
                s2p = [wrk1.tile([P, 1], F32, name=f"s2p{vj}", tag=f"s2p{vj}") for vj in range(4)]
                for vj in range(4):
                    ps3 = psmm.tile([P, 512], F32, space="PSUM", tag="mm")
                    for sk in range(ti + 1):
                        nc.tensor.matmul(
                            out=ps3[:],
                            lhsT=ea2[sk][:, ti * P : (ti + 1) * P],
                            rhs=E[sk][:, vj * 512 : (vj + 1) * 512],
                            start=(sk == 0),
                            stop=(sk == ti),
                        )
                    # drain with free-dim row-sum accumulation (rowsum2 part)
                    if vj % 2 == 0:
                        nc.scalar.activation(
                            out=orow[:, vj * 512 : (vj + 1) * 512],
                            in_=ps3[:],
                            func=mybir.ActivationFunctionType.Copy,
                            accum_out=s2p[vj][:],
                        )
                    else:
                        nc.vector.tensor_scalar(
                            out=orow[:, vj * 512 : (vj + 1) * 512],
                            in0=ps3[:],
                            scalar1=1.0,
                            scalar2=0.0,
                            op0=mybir.AluOpType.mult,
                            op1=mybir.AluOpType.add,
                            accum_out=s2p[vj][:],
                        )
                sAB = wrk1.tile([P, 1], F32, tag="sAB")
                sCD = wrk1.tile([P, 1], F32, tag="sCD")
                nc.vector.tensor_add(out=sAB[:], in0=s2p[0][:], in1=s2p[1][:])
                nc.vector.tensor_add(out=sCD[:], in0=s2p[2][:], in1=s2p[3][:])
                stot = wrk1.tile([P, 1], F32, tag="stot")
                nc.vector.tensor_add(out=stot[:], in0=sAB[:], in1=sCD[:])
                nc.vector.reciprocal(out=rinv2[ti][:], in_=stot[:])
                orow2 = work.tile([P, V], F32, tag="orow2")
                nc.scalar.activation(
                    out=orow2[:],
                    in_=orow[:],
                    func=mybir.ActivationFunctionType.Copy,
                    scale=rinv2[ti][:, :1],
                )
                nc.sync.dma_start(
                    out=out_d[ti * P : (ti + 1) * P, :], in_=orow2[:]
                )

    _nc_cache = nc
    return nc


def _host_a1rawt(v_weight: np.ndarray) -> np.ndarray:
    """Transposed Toeplitz of v_weight with -1e30 on the masked part.

    a1rawT[k, q] = v_weight[q - k] for k <= q, else -1e30. Pure data
    rearrangement of the v_weight input; all arithmetic stays on device.
    """
    v = np.asarray(v_weight, dtype=np.float32).reshape(-1)
    q = np.arange(T)
    relmat = np.clip(q[None, :] - q[:, None], 0, None)
    raw = v[relmat]
    raw[q[:, None] > q[None, :]] = -1e30
    return raw.astype(ml_dtypes.bfloat16)


def kernel(idx, v_weight, W):
    nc = _build()
    idx_np = np.asarray(idx).astype(np.int32)
    w_np = np.ascontiguousarray(
        np.asarray(W, dtype=np.float32).astype(ml_dtypes.bfloat16)
    )
    a1rawt = _host_a1rawt(v_weight)

    in_maps = [
        {
            "idx": np.ascontiguousarray(idx_np[b].reshape(NT, P)),
            "a1rawt": a1rawt,
            "W": w_np,
        }
        for b in range(B)
    ]
    res = run_bass_kernel_spmd(nc, in_maps, list(range(B)))
    return np.stack([np.asarray(res.results[b]["out"]) for b in range(B)], axis=0)
